# revision 7
# baseline (speedup 1.0000x reference)
"""Trainium2 Bass kernel for batched 8-head local-window attention.

Shapes (hardcoded): x [32, 512, 512], w_qkv [512, 1536], w_proj [512, 512],
b_proj [512], mask [1, 1, 512, 512] additive (0 or -1e30).

Strategy: data-parallel over batch across 8 cores (4 batch elements each).
All matmuls in bf16 (fp32 PSUM accumulation).

Tokens are internally permuted to column-major grid order (token' = w*8 + h
for the 8x64 grid) which shrinks the visible query span per 128-key tile
from {384,512,512,384} to {168,208,208,168} columns - roughly halving the
score/exp/mask/attnV work for the 7x11 local window. The permutation is a
pure relabeling applied to x, the mask, and inverted on the output; the
block structure is still derived from the actual mask argument at call
time, so any mask (including dense) works.

Layouts (no transposes anywhere on device):
  - host supplies xT [C, N] per batch
  - qT,kT computed channel-major ([ch, n]) with w_qkv as stationary
  - v computed token-major and stored per (key-tile, head) as a 128-col
    stationary [ones (64) | v_h (64)]
  - S^T = K @ Q^T per head ([key, query-span], key-major); exp on scalar
    engine; 0/1 mask multiply on gpsimd
  - attnV: out^T_h[c, q] accumulated over key tiles with exp(S^T) tiles as
    the MOVING operand (stationary loads stay hidden) into one PSUM bank
    [128, 512] per head; rows 0:64 all hold the softmax denominator
    (thanks to the 64 ones columns) so normalization is one
    reciprocal_approx_fast + one multiply per head, no partition broadcast
  - out^T is channel-major, so the projection consumes it directly as lhsT
"""

import numpy as np
import ml_dtypes

B, N, C = 32, 512, 512
HEADS = 8
HD = C // HEADS
SCALE = HD ** -0.5
NCORES = 8
BPC = B // NCORES  # batches per core
P = 128            # partitions
NT = N // P        # 4 key tiles of 128
CT = C // P        # 4 channel tiles of 128

# column-major grid permutation: position i' = w*8 + h holds token h*64 + w
_H, _W = 8, 64
PERM = np.array([h * _W + w for w in range(_W) for h in range(_H)],
                dtype=np.int64)

_BF16 = ml_dtypes.bfloat16

_cache = {}


def _mask_structure(mask2d):
    """Per key tile t: visible query span [qlo, qlo+w) of the (permuted)
    additive mask [n, m]."""
    vis = mask2d == 0.0  # [n, m] True = visible
    assert vis.any(axis=1).all(), "some query attends to nothing"
    qlo, wid = [], []
    for t in range(NT):
        sub = vis[:, t * P:(t + 1) * P]  # [n, 128]
        rows = np.nonzero(sub.any(axis=1))[0]
        if len(rows) == 0:
            qlo.append(0)
            wid.append(0)
            continue
        qlo.append(int(rows.min()))
        wid.append(int(rows.max()) + 1 - qlo[-1])
    # every query must be covered by at least one tile's span
    cov = np.zeros(N, bool)
    for t in range(NT):
        cov[qlo[t]:qlo[t] + wid[t]] = True
    assert cov.all(), "attnV accumulation would leave psum columns unwritten"
    W = max(wid)
    return W, qlo, wid


def _uniform_groups(entries):
    """Group (t, start, width) entries into runs with equal width and a
    uniform (t, start) stride, so each run is one strided AP op."""
    groups = []
    by_w = {}
    for e in entries:
        if e[2] > 0:
            by_w.setdefault(e[2], []).append(e)
    for w, es in sorted(by_w.items()):
        es = sorted(es)
        while es:
            run = [es[0]]
            for e in es[1:]:
                if len(run) == 1:
                    run.append(e)
                else:
                    d_t = run[1][0] - run[0][0]
                    d_s = run[1][1] - run[0][1]
                    if e[0] - run[-1][0] == d_t and e[1] - run[-1][1] == d_s:
                        run.append(e)
            es = [e for e in es if e not in run]
            groups.append((w, run))
    return groups


def _build(W, qlo, wid):
    import concourse.bass as bass
    import concourse.tile as tile
    import concourse.mybir as mybir
    from concourse import bacc

    fp32 = mybir.dt.float32
    bf16 = mybir.dt.bfloat16
    AF = mybir.ActivationFunctionType

    nc = bacc.Bacc("TRN2", target_bir_lowering=False, debug=False)

    d_xt = nc.dram_tensor("xt", [BPC, C, N], bf16, kind="ExternalInput")
    d_wqkv = nc.dram_tensor("wqkv", [C, 3 * C], bf16, kind="ExternalInput")
    d_wproj = nc.dram_tensor("wproj", [C, C], bf16, kind="ExternalInput")
    d_brep = nc.dram_tensor("brep", [P, C], fp32, kind="ExternalInput")
    d_m01 = nc.dram_tensor("m01", [P, NT, W], bf16, kind="ExternalInput")
    d_y = nc.dram_tensor("y", [BPC, N, C], fp32, kind="ExternalOutput")

    mask_groups = _uniform_groups([(t, 0, wid[t]) for t in range(NT)])
    vis_tiles = [t for t in range(NT) if wid[t] > 0]

    with tile.TileContext(nc) as tc:
        with (
            tc.tile_pool(name="singles", bufs=1) as singles,
            tc.tile_pool(name="xt", bufs=3) as xt_pool,
            tc.tile_pool(name="qk", bufs=2) as qk_pool,
            tc.tile_pool(name="vball", bufs=2) as v_pool,
            tc.tile_pool(name="apair", bufs=3) as a_pool,
            tc.tile_pool(name="oc", bufs=8) as oc_pool,
            tc.tile_pool(name="rcp", bufs=2) as rcp_pool,
            tc.tile_pool(name="ysb", bufs=2) as y_pool,
            tc.tile_pool(name="psS", bufs=4, space="PSUM") as psS_pool,
            tc.tile_pool(name="psB", bufs=2, space="PSUM") as psB_pool,
            tc.tile_pool(name="psO", bufs=2, space="PSUM") as psO_pool,
        ):
            def xt_load(b):
                """Four per-c-tile DMAs so matmuls can start per chunk."""
                xts = []
                for ct in range(CT):
                    x1 = xt_pool.tile([P, N], bf16, tag=f"xt{ct}")
                    nc.sync.dma_start(
                        out=x1, in_=d_xt.ap()[b, ct * P:(ct + 1) * P, :])
                    xts.append(x1)
                return xts

            xts = xt_load(0)
            wq = []
            wq_src = d_wqkv.ap().rearrange("(t p) o -> p t o", p=P)
            for ct in range(CT):
                w1 = singles.tile([P, 3 * C], bf16, tag=f"wqkv{ct}")
                nc.sync.dma_start(out=w1, in_=wq_src[:, ct, :])
                wq.append(w1)

            def load_rest():
                wproj = singles.tile([P, CT, C], bf16)
                nc.sync.dma_start(
                    out=wproj,
                    in_=d_wproj.ap().rearrange("(t p) o -> p t o", p=P))
                m01 = singles.tile([P, NT, W], bf16)
                nc.sync.dma_start(out=m01, in_=d_m01.ap())
                brep = singles.tile([P, C], fp32)
                nc.sync.dma_start(out=brep, in_=d_brep.ap())
                return wproj, m01, brep

            def group_ap(base3d, run, w):
                """AP over [P, len(run), w] from a [P, NT, W] view; `run` is
                [(t, start), ...] with uniform stride."""
                t0, s0 = run[0][0], run[0][1]
                a = base3d[:, t0, s0:s0 + w]
                step = ((run[1][0] - t0) * W + run[1][1] - s0) \
                    if len(run) > 1 else 1
                dims = [a.ap[0], [step, len(run)], [1, w]]
                return bass.AP(tensor=a.tensor, offset=a.offset, ap=dims)

            def qkv_compute(xts, b):
                """qT/kT (channel-major) and v as per-(tile, head) 128-col
                stationaries [v_h | ones]."""
                qk = qk_pool.tile([P, 2 * CT, N], bf16, tag="qk")
                for jj in range(2 * CT):
                    ps = psB_pool.tile([P, N], fp32, tag="psB")
                    for ct in range(CT):
                        nc.tensor.matmul(
                            ps,
                            lhsT=wq[ct][:, jj * P:(jj + 1) * P],
                            rhs=xts[ct],
                            start=(ct == 0), stop=(ct == CT - 1))
                    if jj % 2 == 0:
                        nc.vector.tensor_copy(out=qk[:, jj, :], in_=ps)
                    else:
                        nc.scalar.copy(out=qk[:, jj, :], in_=ps)
                vball = v_pool.tile([P, NT, HEADS, P], bf16, tag="vball")
                # ones columns 0:64 feed the softmax-denominator rows; they
                # sit first so the denominators land at partition base 0
                # (custom-DVE reciprocal requires base-0 operands)
                nc.gpsimd.memset(vball[:, :, :, 0:HD], 1.0)
                for t in range(NT):
                    ps = psB_pool.tile([P, C], fp32, tag="psB")
                    for ct in range(CT):
                        nc.tensor.matmul(
                            ps,
                            lhsT=xts[ct][:, t * P:(t + 1) * P],
                            rhs=wq[ct][:, 2 * C:3 * C],
                            start=(ct == 0), stop=(ct == CT - 1))
                    if t % 2 == 0:
                        nc.vector.tensor_copy(
                            out=vball[:, t, :, HD:P],
                            in_=ps.rearrange("p (h d) -> p h d", h=HEADS))
                    else:
                        nc.scalar.copy(
                            out=vball[:, t, :, HD:P],
                            in_=ps.rearrange("p (h d) -> p h d", h=HEADS))
                return qk, vball

            def score_pair(qk, j):
                """S^T + exp + mask for head pair j -> apair [P, 2, NT, W]."""
                apair = a_pool.tile([P, 2, NT, W], bf16, tag="apair")
                for t in vis_tiles:
                    w = wid[t]
                    for hh in range(2):
                        sl = slice(hh * HD, (hh + 1) * HD)
                        psp = psS_pool.tile([P, N], fp32, tag="psS")
                        nc.tensor.matmul(
                            psp[:, 0:w],
                            lhsT=qk[sl, CT + j, t * P:(t + 1) * P],
                            rhs=qk[sl, j, qlo[t]:qlo[t] + w],
                            start=True, stop=True)
                        nc.scalar.activation(
                            out=apair[:, hh, t, 0:w], in_=psp[:, 0:w],
                            func=AF.Exp)
                for hh in range(2):
                    for w, run in mask_groups:
                        r = [(t, s) for t, s, _ in run]
                        nc.gpsimd.tensor_mul(
                            group_ap(apair[:, hh], r, w),
                            group_ap(apair[:, hh], r, w),
                            group_ap(m01, r, w))
                return apair

            def attnv_pair(apair, vball, j):
                """attn @ [1|v] for heads 2j, 2j+1: out^T accumulated over
                key tiles; rows 0:64 = softmax denominator. Normalize into
                a channel-major bf16 tile [128, N] (proj lhsT)."""
                oc = oc_pool.tile([P, N], bf16, tag="oc")
                for hh in range(2):
                    h = 2 * j + hh
                    pso = psO_pool.tile([P, N], fp32, tag="psO")
                    for i, t in enumerate(vis_tiles):
                        w = wid[t]
                        nc.tensor.matmul(
                            pso[:, qlo[t]:qlo[t] + w],
                            lhsT=vball[:, t, h, :],
                            rhs=apair[:, hh, t, 0:w],
                            start=(i == 0), stop=(i == len(vis_tiles) - 1),
                            skip_group_check=True)
                    rcp = rcp_pool.tile([HD, N], fp32, tag="rcp")
                    nc.vector.reciprocal_approx_fast(rcp, pso[0:HD, :])
                    nc.vector.tensor_mul(
                        oc[hh * HD:(hh + 1) * HD, :], pso[HD:P, :], rcp)
                return oc

            def proj_block(ocs, b, s):
                """Project query block s and stream to DRAM."""
                ps = psB_pool.tile([P, C], fp32, tag="psB")
                for j in range(CT):
                    nc.tensor.matmul(
                        ps,
                        lhsT=ocs[j][:, s * P:(s + 1) * P],
                        rhs=wproj[:, j, :],
                        start=(j == 0), stop=(j == CT - 1))
                ysb = y_pool.tile([P, C], fp32, tag="ysb")
                nc.vector.tensor_add(ysb, ps, brep)
                nc.sync.dma_start(
                    out=d_y.ap()[b, s * P:(s + 1) * P, :], in_=ysb)

            # ---- software-pipelined batch loop ----
            qk, vball = qkv_compute(xts, 0)
            wproj, m01, brep = load_rest()
            xts_pre = xt_load(1) if BPC > 1 else None
            for b in range(BPC):
                apair = score_pair(qk, 0)
                ocs = []
                for j in range(CT):
                    apair_n = score_pair(qk, j + 1) if j + 1 < CT else None
                    ocs.append(attnv_pair(apair, vball, j))
                    apair = apair_n
                if b + 1 < BPC:
                    qk, vball = qkv_compute(xts_pre, b + 1)
                    xts_pre = xt_load(b + 2) if b + 2 < BPC else None
                for s in range(NT):
                    proj_block(ocs, b, s)

    nc.compile()
    return nc


def _prep(x, w_qkv, w_proj, b_proj, mask):
    x = np.asarray(x, np.float32)
    w_qkv = np.asarray(w_qkv, np.float32)
    w_proj = np.asarray(w_proj, np.float32)
    b_proj = np.asarray(b_proj, np.float32)
    mask2d = np.asarray(mask, np.float32).reshape(N, N)
    mask2d = mask2d[PERM][:, PERM]  # column-major token order

    W, qlo, wid = _mask_structure(mask2d)

    ws = w_qkv.copy()
    ws[:, :C] *= SCALE  # fold q scaling into the weights
    wqkv_b = ws.astype(_BF16)
    wproj_b = w_proj.astype(_BF16)
    brep = np.tile(b_proj.reshape(1, C), (P, 1)).astype(np.float32)

    vis = (mask2d == 0.0)
    m01 = np.zeros((P, NT, W), np.float32)
    for t in range(NT):
        # m01[p, t, c] = visible(query qlo[t]+c, key t*128+p)
        w = wid[t]
        m01[:, t, 0:w] = vis[qlo[t]:qlo[t] + w, t * P:(t + 1) * P].T
    m01_b = m01.astype(_BF16)

    # xT per core with permuted tokens: [NCORES, BPC, C, N]
    xp = x[:, PERM, :]
    xt = np.ascontiguousarray(
        xp.reshape(NCORES, BPC, N, C).transpose(0, 1, 3, 2)).astype(_BF16)
    key = (W, tuple(qlo), tuple(wid))
    return xt, wqkv_b, wproj_b, brep, m01_b, key


LAST_RESULTS = None


def kernel(x, w_qkv, w_proj, b_proj, mask, _trace=False):
    global LAST_RESULTS
    from concourse import bass_utils

    xt, wqkv_b, wproj_b, brep, m01_b, key = _prep(
        x, w_qkv, w_proj, b_proj, mask)
    W, qlo, wid = key

    if key not in _cache:
        _cache[key] = _build(W, list(qlo), list(wid))
    nc = _cache[key]

    in_maps = []
    for core in range(NCORES):
        in_maps.append({
            "xt": xt[core],
            "wqkv": wqkv_b,
            "wproj": wproj_b,
            "brep": brep,
            "m01": m01_b,
        })
    res = bass_utils.run_bass_kernel_spmd(
        nc, in_maps, core_ids=list(range(NCORES)), trace=_trace)
    LAST_RESULTS = res
    yp = np.concatenate([res.results[c]["y"] for c in range(NCORES)], axis=0)
    yp = yp.reshape(B, N, C)
    y = np.empty_like(yp)
    y[:, PERM, :] = yp  # undo the token permutation
    return y.astype(np.float32)
